# revision 1
# baseline (speedup 1.0000x reference)
"""GNN message-passing (Convolve) kernel for Trainium2, 8 NeuronCores.

Reference computation (B=8, N=8192, C=256, H=256, O=256, K=64):
    g   = embeddings[:, neighbor_set, :]                     # [B, K, C]
    h   = leaky_relu(g @ Qw + Qb)                            # [B, K, H]
    w   = weights[neighbor_set, node_id]                     # [K]
    s   = sum_k h * w / (sum_k w + eps)                      # [B, H]
    z   = concat(embeddings[:, node_id, :], s)               # [B, C+H]
    o   = leaky_relu(z @ Ww + Wb)                            # [B, O]
    out = o / (||o||_2 + eps)                                # [B, O]

Sharding: data-parallel over the batch axis — core b handles batch b.
Each core receives an augmented table T = [embeddings[b] | weights[:, node_id]]
([N, C+1]) so one indirect-DMA gather fetches both the neighbor embedding
row and its edge weight.  Qw/Ww/biases are replicated.

Device dataflow (fp32):
    constants (identity/ones) built on gpsimd while its DMA library warms
    gather g[64, 257] (one indirect DMA, 16-queue fanout)
    [gather window] node cols via PE transposes; x_p = node.T @ Ww_top
    den_col[64,1] = ones_mat.T @ w_col (+eps, 1/x on DVE) -> wn = w * rec
    h = Prelu(gT.T @ Qw (+ Qb)); s cols = h.T @ wn  (normalized)
    x_p += s_cols.T @ Ww_bot   (same PSUM accumulation group as node part)
    o = Prelu(x_p + Wb); out = o / (sqrt(sum o^2) + eps)   (warm ACT)
"""

import functools

import numpy as np

import concourse.bacc as bacc
import concourse.bass as bass
import concourse.mybir as mybir
import concourse.tile as tile
from concourse.bass_utils import run_bass_kernel_spmd
from concourse.masks import make_identity

B, N, C, H, O, K = 8, 8192, 256, 256, 256, 64
ALPHA = 0.3
EPS = 1e-6
F32 = mybir.dt.float32
I32 = mybir.dt.int32
N_CORES = 8
MULT = mybir.AluOpType.mult
ADD = mybir.AluOpType.add
AF = mybir.ActivationFunctionType


def _build_program(node_id: int, has_qb: bool) -> bass.Bass:
    nc = bacc.Bacc(None, target_bir_lowering=False, debug=False)

    embw = nc.dram_tensor("embw", [N, C + 1], F32, kind="ExternalInput")
    qw = nc.dram_tensor("qw", [C, H], F32, kind="ExternalInput")
    ww = nc.dram_tensor("ww", [C + H, O], F32, kind="ExternalInput")
    wb = nc.dram_tensor("wb", [1, O], F32, kind="ExternalInput")
    nbr = nc.dram_tensor("nbr", [K, 1], I32, kind="ExternalInput")
    if has_qb:
        qb = nc.dram_tensor("qb", [1, H], F32, kind="ExternalInput")
    out_d = nc.dram_tensor("out", [1, O], F32, kind="ExternalOutput")

    with tile.TileContext(nc) as tc:
        with (
            tc.tile_pool(name="sb", bufs=1) as sb,
            tc.tile_pool(name="ps", bufs=1, space="PSUM") as ps,
        ):
            # ---- sync HWDGE: idx first (gates gather), then weights ----
            idx = sb.tile([K, 1], I32)
            nc.sync.dma_start(out=idx[:], in_=nbr[:])
            ww01 = sb.tile([128, 512], F32)
            nc.sync.dma_start(
                out=ww01[:].rearrange("p (two o) -> p two o", two=2),
                in_=ww[0:256, :].rearrange("(two p) o -> p two o", two=2),
            )
            # fused [Qw ; Ww_bot] -> [128, 1024]
            w2 = sb.tile([128, 1024], F32)
            nc.sync.dma_start(
                out=w2[:, 0:512].rearrange("p (two h) -> p two h", two=2),
                in_=qw[:].rearrange("(two p) h -> p two h", two=2),
            )
            nc.sync.dma_start(
                out=w2[:, 512:1024].rearrange("p (two o) -> p two o", two=2),
                in_=ww[256:512, :].rearrange("(two p) o -> p two o", two=2),
            )
            wb_r = sb.tile([1, O], F32)
            nc.sync.dma_start(out=wb_r[:], in_=wb[:])
            # ---- scalar/ACT HWDGE: node row, bias, table warm ----
            cc = sb.tile([1, C], F32)
            nc.scalar.dma_start(out=cc[:], in_=embw[node_id : node_id + 1, 0:C])
            if has_qb:
                qb_r = sb.tile([1, H], F32)
                nc.scalar.dma_start(out=qb_r[:], in_=qb[:])
            warm1 = sb.tile([1, 1], F32)
            nc.scalar.activation(out=warm1[:], in_=cc[0:1, 0:1], func=AF.Square)
            warm2 = sb.tile([1, 1], F32)
            nc.scalar.activation(out=warm2[:], in_=warm1[:], func=AF.Sqrt)

            # ---- gather first: gpsimd's DMA-library ifetch stall starts
            # immediately and overlaps the idx DMA; constants built after ----
            g = sb.tile([K, C + 1], F32)
            nc.gpsimd.indirect_dma_start(
                out=g[:],
                out_offset=None,
                in_=embw[:],
                in_offset=bass.IndirectOffsetOnAxis(ap=idx[:, :1], axis=0),
            )
            # cb = [ eye(64) | ones[64,1] | ones[64,64] ]
            cb = sb.tile([K, 2 * K + 1], F32)
            make_identity(nc, cb[:, 0:K])
            nc.gpsimd.memset(cb[:, K : 2 * K + 1], 1.0)

            # ---- window: node cols; x_p = node.T @ Ww_top (group opens) ----
            z01 = sb.tile([128, 2], F32)
            for j in range(2):
                p = ps.tile([128, 1], F32, tag=f"t{j}")
                nc.tensor.transpose(
                    out=p[:], in_=cc[0:1, 128 * j : 128 * (j + 1)],
                    identity=cb[0:1, 0:1],
                )
                nc.vector.tensor_copy(out=z01[:, j : j + 1], in_=p[:])
            x_p = ps.tile([1, O], F32)
            nc.tensor.matmul(
                out=x_p[:], lhsT=z01[:, 0:1], rhs=ww01[:, 0:256],
                start=True, stop=False, skip_group_check=True,
            )
            nc.tensor.matmul(
                out=x_p[:], lhsT=z01[:, 1:2], rhs=ww01[:, 256:512],
                start=False, stop=False, skip_group_check=True,
            )

            # ---- gT chunks; den_col = ones_mat.T @ w_col ----
            gt = []
            for j in range(2):
                p = ps.tile([128, K], F32, tag=f"t{j}")
                nc.tensor.transpose(
                    out=p[:], in_=g[:, 128 * j : 128 * (j + 1)],
                    identity=cb[:, 0:K],
                )
                s = sb.tile([128, K], F32, tag=f"gts{j}")
                nc.vector.tensor_copy(out=s[:], in_=p[:])
                gt.append(s)
            dc_p = ps.tile([K, 1], F32, tag="t0")
            nc.tensor.matmul(
                out=dc_p[:], lhsT=cb[:, K + 1 : 2 * K + 1], rhs=g[:, C : C + 1],
                start=True, stop=True,
            )
            dc = sb.tile([K, 1], F32)
            nc.vector.tensor_scalar_add(dc[:], dc_p[:], EPS)
            rc = sb.tile([K, 1], F32)
            nc.vector.reciprocal(rc[:], dc[:])
            wn = sb.tile([K, 1], F32)
            nc.vector.tensor_tensor(out=wn[:], in0=g[:, C : C + 1], in1=rc[:], op=MULT)

            # ---- h = Prelu(gT.T @ Qw (+ Qb)) ----
            h_p = ps.tile([K, H], F32)
            nc.tensor.matmul(out=h_p[:], lhsT=gt[0][:], rhs=w2[:, 0:256], start=True, stop=False)
            nc.tensor.matmul(
                out=h_p[:], lhsT=gt[1][:], rhs=w2[:, 256:512],
                start=False, stop=not has_qb,
            )
            if has_qb:
                ones_p = ps.tile([1, K], F32, tag="t1")
                nc.tensor.transpose(out=ones_p[:], in_=cb[:, K : K + 1], identity=cb[:, 0:K])
                ones_r = sb.tile([1, K], F32)
                nc.vector.tensor_copy(out=ones_r[:], in_=ones_p[:])
                nc.tensor.matmul(
                    out=h_p[:], lhsT=ones_r[:], rhs=qb_r[:], start=False, stop=True,
                )
            h_l = sb.tile([K, H], F32)
            nc.scalar.activation(out=h_l[:], in_=h_p[:], func=AF.Prelu, alpha=ALPHA)

            # ---- s cols (normalized) = h.T @ wn; x_p += s.T @ Ww_bot ----
            z23 = sb.tile([128, 2], F32)
            for j in range(2):
                p = ps.tile([128, 1], F32, tag=f"t{j}")
                nc.tensor.matmul(
                    out=p[:], lhsT=h_l[:, 128 * j : 128 * (j + 1)],
                    rhs=wn[:], start=True, stop=True,
                )
                nc.vector.tensor_copy(out=z23[:, j : j + 1], in_=p[:])
            nc.tensor.matmul(
                out=x_p[:], lhsT=z23[:, 0:1], rhs=w2[:, 512:768],
                start=False, stop=False, skip_group_check=True,
            )
            nc.tensor.matmul(
                out=x_p[:], lhsT=z23[:, 1:2], rhs=w2[:, 768:1024],
                start=False, stop=True, skip_group_check=True,
            )

            # ---- o = Prelu(x_p + Wb); out = o/(sqrt(sum o^2)+eps) ----
            x = sb.tile([1, O], F32)
            nc.vector.tensor_tensor(out=x[:], in0=x_p[:], in1=wb_r[:], op=ADD)
            o2 = sb.tile([1, O], F32)
            nc.scalar.activation(out=o2[:], in_=x[:], func=AF.Prelu, alpha=ALPHA)
            sq = sb.tile([1, O], F32)
            n2 = sb.tile([1, 1], F32)
            nc.scalar.activation(out=sq[:], in_=o2[:], func=AF.Square, accum_out=n2[:])
            nrm = sb.tile([1, 1], F32)
            nc.scalar.activation(out=nrm[:], in_=n2[:], func=AF.Sqrt)
            den2 = sb.tile([1, 1], F32)
            nc.vector.tensor_scalar_add(den2[:], nrm[:], EPS)
            rec2 = sb.tile([1, 1], F32)
            nc.vector.reciprocal(rec2[:], den2[:])
            res = sb.tile([1, O], F32)
            nc.vector.tensor_scalar_mul(res[:], o2[:], rec2[:])

            nc.sync.dma_start(out=out_d[:], in_=res[:])

    nc.finalize()
    return nc


@functools.lru_cache(maxsize=4)
def _program(node_id: int, has_qb: bool) -> bass.Bass:
    return _build_program(node_id, has_qb)


def kernel(
    embeddings: np.ndarray,
    weights: np.ndarray,
    Qw: np.ndarray,
    Qb: np.ndarray,
    Ww: np.ndarray,
    Wb: np.ndarray,
    neighbor_set: np.ndarray,
    node_id,
    _trace: bool = False,
):
    node_id = int(np.asarray(node_id))
    nbr = np.ascontiguousarray(
        np.asarray(neighbor_set).astype(np.int32).reshape(K, 1)
    )
    wcol = np.asarray(weights[:, node_id], dtype=np.float32).reshape(N, 1)
    qw = np.ascontiguousarray(Qw, dtype=np.float32)
    qb = np.ascontiguousarray(Qb, dtype=np.float32).reshape(1, H)
    ww = np.ascontiguousarray(Ww, dtype=np.float32)
    wb = np.ascontiguousarray(Wb, dtype=np.float32).reshape(1, O)
    has_qb = bool(np.any(qb))

    nc = _program(node_id, has_qb)
    in_maps = []
    for b in range(N_CORES):
        m = {
            "embw": np.concatenate(
                [np.asarray(embeddings[b], dtype=np.float32), wcol], axis=1
            ),
            "qw": qw,
            "ww": ww,
            "wb": wb,
            "nbr": nbr,
        }
        if has_qb:
            m["qb"] = qb
        in_maps.append(m)
    r = run_bass_kernel_spmd(nc, in_maps, list(range(N_CORES)), trace=_trace)
    out = np.stack([r.results[b]["out"][0] for b in range(N_CORES)], axis=0)
    if _trace:
        return out, r
    return out

